# revision 1
# baseline (speedup 1.0000x reference)
# Tensor-parallel fused attention kernel for TRN2, 8 cores.
# Core r owns heads {2r, 2r+1}. Inputs per core:
#   x     [2*S, 1024] f32  (replicated; rows = b*S + s)
#   wqkv  [1024, 512] f32  (columns: q_h1|q_h2|k_h1|k_h2|v_h1|v_h2|g_h1|g_h2, 64 each)
#   wout  [1024, 128] f32  (w_out columns 128r:128r+128)
#   freqs [S, 32] f32      (replicated)
#   lnp   [8, 64] f32      ([qn_w, qn_w, kn_w, kn_w, qn_b, qn_b, kn_b, kn_b])
#   sel2  [2, 128] f32     (recip-broadcast selector constant)
# Output per core:
#   out   [128, 2*S] f32   (out^T slice: rows = w_out columns owned by this core)
import math

import concourse.bass as bass
import concourse.mybir as mybir
from concourse import bacc, tile

F32 = mybir.dt.float32
F16 = mybir.dt.float16
AF = mybir.ActivationFunctionType
ALU = mybir.AluOpType
AX = mybir.AxisListType

DIM = 1024
HD = 64
EPS = 1e-5


def build(S: int, n_cores: int = 8, reps: int = 1, apply_ln_affine: bool = False,
          dma_denom: bool = False):
    TB = S // 128            # t-tiles per batch
    TT = 2 * TB
    JT = S // 64             # kj 64-blocks per batch
    QW = min(512, S)
    QT = S // QW             # qi tiles per batch
    JPE = 2 if S >= 128 else 1   # J tiles per exp op

    nc = bacc.Bacc("TRN2", target_bir_lowering=False, debug=False, num_devices=n_cores)

    X = nc.dram_tensor("x", [2 * S, DIM], F32, kind="ExternalInput")
    WQKV = nc.dram_tensor("wqkv", [DIM, 512], F32, kind="ExternalInput")
    WOUT = nc.dram_tensor("wout", [DIM, 128], F32, kind="ExternalInput")
    FREQS = nc.dram_tensor("freqs", [S, 32], F32, kind="ExternalInput")
    LNP = nc.dram_tensor("lnp", [8, HD], F32, kind="ExternalInput")
    SEL2 = nc.dram_tensor("sel2", [2, 128], F32, kind="ExternalInput")
    OUT = nc.dram_tensor("out", [128, 2 * S], F32, kind="ExternalOutput")

    ag_in = [nc.dram_tensor(f"ag_in{b}", [128, S], F16) for b in range(2)]
    ag_out = [nc.dram_tensor(f"ag_out{b}", [8 * 128, S], F16, addr_space="Shared")
              for b in range(2)]

    with tile.TileContext(nc) as tc:
        with (
            tc.tile_pool(name="persist", bufs=1) as pp,
            tc.tile_pool(name="work", bufs=2) as wp,
            tc.tile_pool(name="espool", bufs=4) as ep,
            tc.tile_pool(name="xload", bufs=3) as xp,
            tc.tile_pool(name="small", bufs=2) as sp,
        ):
            # ---- constants & weights ----
            w16 = pp.tile([128, 8, 512], F16, tag="w16")
            w16o = pp.tile([128, 8, 128], F16, tag="w16o")
            for c in range(8):
                w32 = wp.tile([128, 512], F32, tag="wld")
                nc.gpsimd.dma_start(w32[:], WQKV[c * 128:(c + 1) * 128, :])
                nc.vector.tensor_copy(w16[:, c, :], w32[:])
                wo32 = wp.tile([128, 128], F32, tag="wold")
                nc.gpsimd.dma_start(wo32[:], WOUT[c * 128:(c + 1) * 128, :])
                nc.vector.tensor_copy(w16o[:, c, :], wo32[:])

            if apply_ln_affine:
                lnp1 = sp.tile([1, 512], F32, tag="lnp1")
                nc.gpsimd.dma_start(
                    lnp1[:], LNP.ap().rearrange("a b -> (a b)").unsqueeze(0))
                ones1 = sp.tile([1, 128], F32, tag="ones1")
                nc.vector.memset(ones1[:], 1.0)
                with tc.tile_pool(name="pbc", bufs=1, space="PSUM") as pbc:
                    lnb_ps = pbc.tile([128, 512], F32)
                    nc.tensor.matmul(lnb_ps[:], ones1[:], lnp1[:], start=True, stop=True)
                    lnwb = pp.tile([128, 512], F32, tag="lnwb")
                    nc.scalar.copy(lnwb[:], lnb_ps[:])

            # cos/sin tables [128, TB*32]
            ftile = sp.tile([128, TB * 32], F32, tag="ftile")
            nc.gpsimd.dma_start(
                ftile[:].rearrange("p (a c) -> p a c", c=32),
                bass.AP(FREQS.ap().tensor, 0, [[32, 128], [128 * 32, TB], [1, 32]]),
            )
            sin_t = pp.tile([128, TB * 32], F32, tag="sin_t")
            cos_t = pp.tile([128, TB * 32], F32, tag="cos_t")
            halfpi = sp.tile([128, 1], F32, tag="halfpi")
            nc.vector.memset(halfpi[:], math.pi / 2)
            epsc = pp.tile([128, 1], F32, tag="epsc")
            nc.vector.memset(epsc[:], EPS)
            nc.scalar.activation(sin_t[:], ftile[:], AF.Sin)
            nc.scalar.activation(cos_t[:], ftile[:], AF.Sin, bias=halfpi[:])

            ones_bd = pp.tile([128, 2], F16, tag="ones_bd")
            nc.vector.memset(ones_bd[:], 0.0)
            nc.vector.memset(ones_bd[0:64, 0:1], 1.0)
            nc.vector.memset(ones_bd[64:128, 1:2], 1.0)
            ones_bd32 = pp.tile([128, 2], F32, tag="ones_bd32")
            nc.vector.memset(ones_bd32[:], 0.0)
            nc.vector.memset(ones_bd32[0:64, 0:1], 1.0)
            nc.vector.memset(ones_bd32[64:128, 1:2], 1.0)
            sel2 = pp.tile([2, 128], F32, tag="sel2")
            nc.gpsimd.dma_start(sel2[:], SEL2[:])

            # ---- persistent activations ----
            qkgT = pp.tile([128, TT, 3, 128], F16, tag="qkgT")
            og = pp.tile([128, 2 * S], F16, tag="og")
            k_bd = pp.tile([128, JT, 128], F16, tag="k_bd")
            v_bd = pp.tile([128, 2 * JT, 128], F16, tag="v_bd")
            nc.vector.memset(k_bd[:], 0.0)
            nc.vector.memset(v_bd[:], 0.0)

            for _rep in range(reps):
              with (
                tc.tile_pool(name="ph1", bufs=1) as p1,
                tc.tile_pool(name="ps_a", bufs=2, space="PSUM") as ps_a,
                tc.tile_pool(name="ps_s", bufs=2, space="PSUM") as ps_s,
                tc.tile_pool(name="ps_o", bufs=1, space="PSUM") as ps_o,
                tc.tile_pool(name="ps_d", bufs=1, space="PSUM") as ps_d,
              ):
                xc_all = p1.tile([128, TT, 256], F32, tag="xc_all")
                qkg16 = p1.tile([128, TT, 384], F16, tag="qkg16")
                negm = p1.tile([128, TT * 4], F32, tag="negm")
                rstd = p1.tile([128, TT * 4], F32, tag="rstd")
                ssq = p1.tile([128, TT * 4], F32, tag="ssq")
                t1 = p1.tile([128, 1024], F32, tag="t1")
                t2 = p1.tile([128, 1024], F32, tag="t2")

                for b in range(2):
                    # ---- phase 1a(b): qkv matmul, mean-sub evac, v_bd, gate ----
                    for T in range(b * TB, (b + 1) * TB):
                        xt16 = xp.tile([128, DIM], F16, tag="xt16")
                        nc.gpsimd.dma_start(xt16[:], X[T * 128:(T + 1) * 128, :])
                        xT16 = xp.tile([128, 8, 128], F16, tag="xT16")
                        nc.sync.dma_start_transpose(xT16[:], xt16[:])

                        psq = ps_a.tile([128, 512], F32, tag="psa")
                        for c in range(8):
                            nc.tensor.matmul(psq[:], xT16[:, c, :], w16[:, c, :],
                                             start=(c == 0), stop=(c == 7))

                        qk_ps = psq[:, 0:256].rearrange("p (a b) -> p a b", b=HD)
                        nm = negm[:, T * 4:(T + 1) * 4]
                        nc.vector.tensor_reduce(nm, qk_ps, AX.X, ALU.add)
                        nc.vector.tensor_scalar_mul(nm, nm, -1.0 / HD)
                        nc.vector.tensor_tensor(
                            xc_all[:, T, :].rearrange("p (a b) -> p a b", b=HD),
                            qk_ps,
                            nm.unsqueeze(2).broadcast_to([128, 4, HD]), ALU.add)
                        J0 = 2 * (T % TB)
                        vb = v_bd[:, b * JT:(b + 1) * JT, :]
                        nc.vector.tensor_copy(vb[0:64, J0, 0:64], psq[0:64, 256:320])
                        nc.vector.tensor_copy(vb[64:128, J0, 64:128], psq[0:64, 320:384])
                        nc.vector.tensor_copy(vb[0:64, J0 + 1, 0:64], psq[64:128, 256:320])
                        nc.vector.tensor_copy(vb[64:128, J0 + 1, 64:128],
                                              psq[64:128, 320:384])
                        nc.scalar.activation(qkg16[:, T, 256:384], psq[:, 384:512],
                                             AF.Sigmoid)

                    # ---- phase 1b(b): LN scale + rope (chunked) ----
                    CH = min(4, TB)
                    for ci in range(TB // CH):
                        T0 = b * TB + ci * CH
                        xcb = xc_all[:, T0:T0 + CH, :]
                        nc.vector.tensor_tensor(
                            t1[:, 0:CH * 256].rearrange("p (a c) -> p a c", c=256),
                            xcb, xcb, ALU.mult)
                        nc.vector.tensor_reduce(
                            ssq[:, T0 * 4:(T0 + CH) * 4],
                            t1[:, 0:CH * 256].rearrange("p (a c) -> p a c", c=HD),
                            AX.X, ALU.add)
                    sqv = ssq[:, b * TB * 4:(b + 1) * TB * 4]
                    rsv = rstd[:, b * TB * 4:(b + 1) * TB * 4]
                    nc.scalar.activation(sqv, sqv, AF.Sqrt, scale=1.0 / HD,
                                         bias=epsc[:])
                    nc.vector.reciprocal(rsv, sqv)
                    xcb = xc_all[:, b * TB:(b + 1) * TB, :]
                    nc.vector.tensor_tensor(
                        xcb.rearrange("p a (s d) -> p (a s) d", d=HD),
                        xcb.rearrange("p a (s d) -> p (a s) d", d=HD),
                        rsv.unsqueeze(2).broadcast_to([128, TB * 4, HD]), ALU.mult)
                    if apply_ln_affine:
                        nc.vector.tensor_tensor(
                            xcb, xcb,
                            lnwb[:, 0:256].unsqueeze(1).broadcast_to([128, TB, 256]),
                            ALU.mult)
                        nc.vector.tensor_tensor(
                            xcb, xcb,
                            lnwb[:, 256:512].unsqueeze(1).broadcast_to([128, TB, 256]),
                            ALU.add)
                    RH = min(8, TB)
                    for ci in range(TB // RH):
                        T0 = b * TB + ci * RH
                        st0 = (ci * RH) * 32
                        xt = xc_all.tensor
                        base = xc_all[:, T0, :].offset
                        pstep = xc_all[:].ap[0][0]
                        xe = bass.AP(xt, base, [[pstep, 128], [256, RH], [HD, 4], [2, 32]])
                        xo = bass.AP(xt, base + 1, [[pstep, 128], [256, RH], [HD, 4], [2, 32]])
                        qt_ = qkg16.tensor
                        qbase = qkg16[:, T0, 0].offset
                        qstep = qkg16[:].ap[0][0]
                        qe = bass.AP(qt_, qbase, [[qstep, 128], [384, RH], [HD, 4], [2, 32]])
                        qo = bass.AP(qt_, qbase + 1, [[qstep, 128], [384, RH], [HD, 4], [2, 32]])
                        cstep = cos_t[:].ap[0][0]
                        cosb = bass.AP(cos_t.tensor, cos_t[:].offset + st0,
                                       [[cstep, 128], [32, RH], [0, 4], [1, 32]])
                        sinb = bass.AP(sin_t.tensor, sin_t[:].offset + st0,
                                       [[cstep, 128], [32, RH], [0, 4], [1, 32]])
                        t13 = t1[:, 0:RH * 128].rearrange("p (a s c) -> p a s c", s=4, c=32)
                        t23 = t2[:, 0:RH * 128].rearrange("p (a s c) -> p a s c", s=4, c=32)
                        nc.vector.tensor_tensor(t13, xe, cosb, ALU.mult)
                        nc.vector.tensor_tensor(t23, xo, sinb, ALU.mult)
                        nc.vector.tensor_tensor(qe, t13, t23, ALU.subtract)
                        nc.vector.tensor_tensor(t13, xe, sinb, ALU.mult)
                        nc.vector.tensor_tensor(t23, xo, cosb, ALU.mult)
                        nc.vector.tensor_tensor(qo, t13, t23, ALU.add)

                    # ---- phase 1c(b): q|k|g transposes ----
                    for T in range(b * TB, (b + 1) * TB):
                        nc.sync.dma_start_transpose(qkgT[:, T, :, :], qkg16[:, T, :])

                    # ---- phase 2(b): attention ----
                    for J in range(JT):
                        Tl = b * TB + J // 2
                        off = 64 * (J % 2)
                        nc.vector.tensor_copy(k_bd[0:64, J, 0:64],
                                               qkgT[0:64, Tl, 1, off:off + 64])
                        nc.vector.tensor_copy(k_bd[64:128, J, 64:128],
                                               qkgT[64:128, Tl, 1, off:off + 64])
                    for Q in range(QT):
                        nq = QW // 128
                        qs2 = qkgT[:, b * TB + Q * nq: b * TB + (Q + 1) * nq, 0, :]
                        po = ps_o.tile([128, QW], F32, tag="po")
                        pd = ps_d.tile([2, QW], F32, tag="pd")
                        if dma_denom:
                            acc32 = sp.tile([128, QW], F32, tag="acc32")
                        for Jb in range(JT // JPE):
                            ps = ps_s.tile([128, JPE * QW], F32, tag="ps")
                            for jj in range(JPE):
                                J = Jb * JPE + jj
                                nc.tensor.matmul(ps[:, jj * QW:(jj + 1) * QW],
                                                 k_bd[:, J, :], qs2,
                                                 start=True, stop=True)
                            es = ep.tile([128, JPE * QW], F16, tag="es")
                            nc.scalar.activation(es[:], ps[:], AF.Exp, scale=0.125)
                            for jj in range(JPE):
                                J = Jb * JPE + jj
                                esj = es[:, jj * QW:(jj + 1) * QW]
                                nc.tensor.matmul(po[:], v_bd[:, b * JT + J, :], esj,
                                                 start=(J == 0), stop=(J == JT - 1),
                                                 skip_group_check=True)
                                if dma_denom:
                                    nc.gpsimd.dma_start(
                                        acc32[:], esj,
                                        accum_op=(ALU.bypass if J == 0 else ALU.add))
                                else:
                                    nc.tensor.matmul(pd[:], ones_bd[:], esj,
                                                     start=(J == 0), stop=(J == JT - 1),
                                                     skip_group_check=True)
                        rd = sp.tile([2, QW], F32, tag="rd")
                        if dma_denom:
                            nc.tensor.matmul(pd[:], ones_bd32[:], acc32[:],
                                             start=True, stop=True)
                        nc.vector.reciprocal(rd[:], pd[:])
                        pr = ps_s.tile([128, JPE * QW], F32, tag="ps")
                        prv = pr[:, 0:QW]
                        nc.tensor.matmul(prv, sel2[:], rd[:], start=True, stop=True)
                        r32 = wp.tile([128, QW], F32, tag="r32")
                        nc.vector.tensor_copy(r32[:], prv)
                        on = wp.tile([128, QW], F32, tag="on")
                        nc.vector.tensor_tensor(on[:], po[:], r32[:], ALU.mult)
                        gs = qkgT[:, b * TB + Q * nq: b * TB + (Q + 1) * nq, 2, :]
                        gq = b * QT + Q
                        nc.vector.tensor_tensor(
                            og[:, gq * QW:(gq + 1) * QW].rearrange(
                                "p (a b) -> p a b", b=128),
                            on[:].rearrange("p (a b) -> p a b", b=128),
                            gs, ALU.mult)

                for b in range(2):
                    # ---- phase 3(b): all-gather + out projection ----
                    nc.gpsimd.dma_start(ag_in[b].ap(), og[:, b * S:(b + 1) * S])
                    nc.gpsimd.collective_compute(
                        "AllGather", ALU.bypass,
                        replica_groups=[list(range(n_cores))],
                        ins=[ag_in[b].ap()], outs=[ag_out[b].ap()],
                    )
                    OW = min(512, S)
                    for TTi in range(S // OW):
                        pot = ps_a.tile([128, 512], F32, tag="psa")
                        potv = pot[:, 0:OW]
                        for c in range(8):
                            ogf = wp.tile([128, OW], F16, tag="ogf")
                            nc.scalar.dma_start(
                                ogf[:], ag_out[b][c * 128:(c + 1) * 128,
                                                  TTi * OW:(TTi + 1) * OW])
                            nc.tensor.matmul(potv, w16o[:, c, :], ogf[:],
                                             start=(c == 0), stop=(c == 7))
                        ot32 = wp.tile([128, OW], F32, tag="ot32")
                        nc.scalar.copy(ot32[:], potv)
                        nc.gpsimd.dma_start(
                            OUT[:, b * S + TTi * OW: b * S + (TTi + 1) * OW],
                            ot32[:])

    nc.compile()
    return nc


def shard_inputs(x, freqs, w_qkv, w_out, qn_w, qn_b, kn_w, kn_b, n_cores=8):
    import numpy as np
    B, S, _ = x.shape
    x2 = np.ascontiguousarray(x.reshape(2 * S, DIM), dtype=np.float32)
    lnp_base = np.stack([qn_w, qn_w, kn_w, kn_w, qn_b, qn_b, kn_b, kn_b]).astype(np.float32)
    sel2c = np.zeros((2, 128), np.float32)
    sel2c[0, 0:64] = 1.0
    sel2c[1, 64:128] = 1.0
    maps = []
    for r in range(n_cores):
        cols = []
        for sec in range(4):
            c0 = sec * DIM + 128 * r
            cols.append(w_qkv[:, c0:c0 + 128])
        wq = np.ascontiguousarray(np.concatenate(cols, axis=1), dtype=np.float32)
        wo = np.ascontiguousarray(w_out[:, 128 * r:128 * (r + 1)], dtype=np.float32)
        maps.append({
            "x": x2, "wqkv": wq, "wout": wo,
            "freqs": np.ascontiguousarray(freqs, dtype=np.float32),
            "lnp": lnp_base, "sel2": sel2c,
        })
    return maps


def unshard_output(results, S):
    import numpy as np
    outT = np.concatenate([r["out"] for r in results], axis=0)  # [1024, 2S]
    return np.ascontiguousarray(outT.T).reshape(2, S, DIM)


_NC_CACHE = {}


def _get_nc(S, affine):
    key = (S, affine)
    if key not in _NC_CACHE:
        _NC_CACHE[key] = build(S, apply_ln_affine=affine)
    return _NC_CACHE[key]


def kernel(x, freqs, w_qkv, w_out, qn_w, qn_b, kn_w, kn_b):
    """Full-input entrypoint: shards across 8 neuron cores, runs, gathers."""
    import numpy as np
    from concourse.bass_utils import run_bass_kernel_spmd

    x = np.asarray(x, dtype=np.float32)
    freqs = np.asarray(freqs, dtype=np.float32)
    w_qkv = np.asarray(w_qkv, dtype=np.float32)
    w_out = np.asarray(w_out, dtype=np.float32)
    qn_w, qn_b = np.asarray(qn_w), np.asarray(qn_b)
    kn_w, kn_b = np.asarray(kn_w), np.asarray(kn_b)
    B, S, _ = x.shape
    affine = not (np.all(qn_w == 1) and np.all(qn_b == 0)
                  and np.all(kn_w == 1) and np.all(kn_b == 0))
    nc = _get_nc(S, bool(affine))
    maps = shard_inputs(x, freqs, w_qkv, w_out, qn_w, qn_b, kn_w, kn_b)
    res = run_bass_kernel_spmd(nc, maps, list(range(8)))
    return unshard_output(res.results, S)

